# revision 1
# baseline (speedup 1.0000x reference)
"""DUQ RBF head kernel for Trainium2 (8 NeuronCores, batch-parallel).

Computes out[b,c,h,w] = exp(gamma * mean_e (einsum('bfhw,ecf', x, W) - m/N)^2)
for features [8,512,128,128], weights [16,64,512], m [16,64], N [64].

Strategy: data-parallel over batch (1 image per core). Per core, one big
matmul [ec=1024, f=512] @ [f=512, pix=16384] done as 8 ec-chunks x 32
pixel-tiles x 4 K-chunks on the tensor engine in float32r (full-rate fp32),
centroid subtraction folded into the per-partition bias of an ACT Square
epilogue, DVE accumulation over ec-chunks, final ACT Exp. Built on Bacc so
multi-wait instructions are legalized (split to event semaphores).
"""

import numpy as np

import concourse.bacc as bacc_mod
import concourse.mybir as mybir
import concourse.tile as tile
from concourse.bass_utils import run_bass_kernel_spmd

dt = mybir.dt
Act = mybir.ActivationFunctionType

B, F, H, W = 8, 512, 128, 128
E, C = 16, 64
PIX = H * W          # 16384 pixels per image
NT = 512             # pixel tile (psum free dim)
NTILES = PIX // NT   # 32
MCH = (E * C) // 128  # 8 ec-chunks of 128 partitions
KCH = F // 128        # 4 contraction chunks
LENGTH_SCALE = 0.1
GAMMA = -1.0 / (2.0 * LENGTH_SCALE**2)   # -50.0
EXP_SCALE = GAMMA / E                    # -3.125


def _build():
    nc = bacc_mod.Bacc(None)
    feat_d = nc.declare_dram_parameter("feat", [F, PIX], dt.float32r, isOutput=False)
    wt_d = nc.declare_dram_parameter("wt", [F, E * C], dt.float32r, isOutput=False)
    negc_d = nc.declare_dram_parameter("negc", [128, MCH], dt.float32, isOutput=False)
    out_d = nc.declare_dram_parameter("out", [C, PIX], dt.float32, isOutput=True)

    feat_k = feat_d.rearrange("(k p) x -> p k x", k=KCH)

    with tile.TileContext(nc) as tc:
        with (
            tc.tile_pool(name="singles", bufs=1) as singles,
            tc.tile_pool(name="xin", bufs=6) as xin,
            tc.tile_pool(name="sqp", bufs=3) as sqp,
            tc.tile_pool(name="accp", bufs=3) as accp,
            tc.tile_pool(name="outp", bufs=4) as outp,
            tc.tile_pool(name="ps", bufs=8, space="PSUM") as ps,
        ):
            wt_k = wt_d.rearrange("(k p) m -> p k m", k=KCH)
            ws = []
            for m in range(MCH):
                wsm = singles.tile([128, KCH, 128], dt.float32r, tag=f"ws{m}")
                nc.sync.dma_start(
                    out=wsm, in_=wt_k[:, :, m * 128 : (m + 1) * 128]
                )
                ws.append(wsm)
            negc_sb = singles.tile([128, MCH], dt.float32, tag="negc")
            nc.sync.dma_start(out=negc_sb, in_=negc_d[:, :])

            for t in range(NTILES):
                px = slice(t * NT, (t + 1) * NT)
                xt = []
                for k in range(KCH):
                    xtk = xin.tile([128, NT], dt.float32r, tag=f"x{k}")
                    nc.sync.dma_start(out=xtk, in_=feat_k[:, k, px])
                    xt.append(xtk)

                acc = accp.tile([128, NT], dt.float32, tag="acc")
                for m in range(MCH):
                    pst = ps.tile([128, NT], dt.float32, tag="mm")
                    for k in range(KCH):
                        nc.tensor.matmul(
                            out=pst, lhsT=ws[m][:, k, :], rhs=xt[k],
                            start=(k == 0), stop=(k == KCH - 1),
                        )
                    if m == 0:
                        nc.scalar.activation(
                            out=acc, in_=pst, func=Act.Square,
                            bias=negc_sb[:, 0:1], scale=1.0,
                        )
                    else:
                        sq = sqp.tile([128, NT], dt.float32, tag="sq")
                        nc.scalar.activation(
                            out=sq, in_=pst, func=Act.Square,
                            bias=negc_sb[:, m : m + 1], scale=1.0,
                        )
                        nc.vector.tensor_add(out=acc, in0=acc, in1=sq)

                tmp = outp.tile([64, NT], dt.float32, tag="tmp")
                nc.vector.tensor_copy(out=tmp, in_=acc[64:128, :])
                hc = outp.tile([64, NT], dt.float32, tag="hc")
                nc.vector.tensor_add(out=hc, in0=acc[0:64, :], in1=tmp)
                eo = outp.tile([64, NT], dt.float32, tag="eo")
                nc.scalar.activation(
                    out=eo, in_=hc, func=Act.Exp, bias=0.0, scale=EXP_SCALE
                )
                nc.scalar.dma_start(out=out_d[:, px], in_=eo)

    nc.finalize()
    return nc


_NC_CACHE = {}


def _get_nc():
    if "nc" not in _NC_CACHE:
        _NC_CACHE["nc"] = _build()
    return _NC_CACHE["nc"]


def _prep_inputs(features, weights, m, N):
    # wt[f, e*64+c] = weights[e, c, f]
    wt = np.ascontiguousarray(
        weights.astype(np.float32).transpose(2, 0, 1).reshape(F, E * C)
    )
    cent = (m.astype(np.float32) / N.astype(np.float32)[None, :]).reshape(-1)  # [ec]
    negc = np.ascontiguousarray(-cent.reshape(MCH, 128).T)  # [128, MCH]
    feats = np.ascontiguousarray(features.astype(np.float32).reshape(B, F, PIX))
    return [{"feat": feats[i], "wt": wt, "negc": negc} for i in range(B)]


def run_spmd(features, weights, m, N, trace=False):
    in_maps = _prep_inputs(features, weights, m, N)
    res = run_bass_kernel_spmd(_get_nc(), in_maps, list(range(B)), trace=trace)
    out = np.stack([res.results[i]["out"] for i in range(B)])  # [B, C, PIX]
    return out.reshape(B, C, H, W).astype(np.float32), res


def kernel(features, weights, m, N):
    out, _ = run_spmd(features, weights, m, N, trace=False)
    return out

